# revision 50
# baseline (speedup 1.0000x reference)
"""Trainium2 Bass kernel for nn_BaseTimeAttention (dense transformer block:
QKV projection + RoPE + softmax attention + output projection).

Problem (hardcoded):
  x:  [B=2, S=2048, H=2048] fp32,  Wq/Wk/Wv/Wo: [2048, 2048] fp32
  out = softmax((rope(xWq^T) rope(xWk^T)^T)/sqrt(128)) (xWv^T) Wo^T

Sharding (8 cores): tensor-parallel over heads x data-parallel over batch.
Core c handles batch b=c//4 and head group g=c%4 (4 of 16 heads = 512 of 2048
channels). Each core produces a full [2048, 2048] partial of the output
projection restricted to its 512 input channels; the host sums 4 partials per
batch (o_proj row-parallel reduction on host).

v2 design (vs the fp32r baseline at 532us):
  * Everything flows in bf16 except PSUM accumulation and the final output.
    Halves all DMA traffic and lets q/k/v stay SBUF-resident for the whole
    kernel -- no DRAM round trip between projection and attention, so the
    phase transitions have no DMA stall.
  * Softmax denominator: instead of 16 ones-matmuls per (head, q-block)
    (256 matmuls, ~60us of PE), the exp tiles are accumulated on the Vector
    engine (bf16 adds) and a single ones-matmul per block contracts the
    128 partitions. The den matmul + normalize for block B is emitted at the
    start of block B+1 so the PE never waits on the DVE accumulation chain.
  * Startup DMAs are ordered so the first matmul can start ~2us in: first
    weight chunk on the sync ring || first x chunk then per-block cos/sin
    slices on the scalar ring.

Per-core PE work: 768 (proj) + 256 (scores) + 256 (attn@V) + 16 (den) +
256 (o_proj) = 1552 matmuls of 512 moving rows = 795k cycles ~= 331us at
2.4GHz; phase 2 is co-limited by ScalarE exp (~140us for the 16.8M scores).
"""

import numpy as np

import concourse.mybir as mybir
import concourse.tile as tile
from concourse import bacc
from concourse.bass_utils import run_bass_kernel_spmd

F32 = mybir.dt.float32
BF16 = mybir.dt.bfloat16
F8 = mybir.dt.float8e4
AF = mybir.ActivationFunctionType

B = 2
S = 2048
HIDDEN = 2048
HEADS = 16
DH = 128
THETA = 10000.0
N_CORES = 8
GROUPS = 4
HPC = HEADS // GROUPS  # heads per core
JPC = HPC * DH  # projection cols per core
SCALE = 1.0 / np.sqrt(DH)

SB = 512
NSB = S // SB
KT = HIDDEN // 128  # 16 contraction tiles
NKT = S // 128  # 16 s_k tiles


def build():
    nc = bacc.Bacc("TRN2", target_bir_lowering=False, debug=False)

    # partition-major inputs (see _make_in_maps)
    x_d = nc.dram_tensor("xPM", [NSB, 128, KT, SB], BF16, kind="ExternalInput")
    wq_d = nc.dram_tensor("wqPM", [128, KT, JPC], BF16, kind="ExternalInput")
    wk_d = nc.dram_tensor("wkPM", [128, KT, JPC], BF16, kind="ExternalInput")
    wv_d = nc.dram_tensor("wvPM", [128, KT, JPC], BF16, kind="ExternalInput")
    wo_d = nc.dram_tensor("woPM", [128, HPC, HIDDEN], BF16, kind="ExternalInput")
    cos_d = nc.dram_tensor("cos", [DH, S], BF16, kind="ExternalInput")
    sin_d = nc.dram_tensor("sinS", [DH, S], BF16, kind="ExternalInput")
    out_d = nc.dram_tensor("out", [S, HIDDEN], F32, kind="ExternalOutput")

    out = out_d.ap()

    with tile.TileContext(nc) as tc:
        with tc.tile_pool(name="persist", bufs=1) as persist:
            ones_sb = persist.tile([128, 128], BF16, tag="ones")
            nc.gpsimd.memset(ones_sb[:], 1.0)

            # SBUF-resident per-head q/k (transposed [dh, s]) and natural v
            qh = [
                persist.tile([128, S], BF16, tag=f"qh{h}", name=f"qh{h}")
                for h in range(HPC)
            ]
            kh = [
                persist.tile([128, S], BF16, tag=f"kh{h}", name=f"kh{h}")
                for h in range(HPC)
            ]
            vnat = [
                persist.tile([128, JPC], BF16, tag=f"v{t}", name=f"v{t}")
                for t in range(NKT)
            ]
            cos_sb = persist.tile([128, S], BF16, tag="cos")
            sin_sb = persist.tile([128, S], BF16, tag="sin")
            wo = persist.tile([128, HPC, HIDDEN], BF16, tag="wo")
            yt = persist.tile([128, HPC, S], BF16, tag="yt")

            # ---------------- Phase 1: projections + RoPE ------------------
            # s-block outer, projection inner: x is streamed from HBM once.
            # Chunked loads (finest first) so the first matmuls start as soon
            # as the DMA rings deliver data (~10us fixed startup latency).
            NCH = 5
            CHS = (1, 1, 2, 4, 8)  # weight chunk sizes
            CHO = (0, 1, 2, 4, 8)  # weight chunk offsets
            NCHX = 6
            CHSX = (1, 1, 2, 4, 4, 4)  # x chunk sizes (k8-11 ride sync ring)
            CHOX = (0, 1, 2, 4, 8, 12)

            def wslice(chunks, k, cols, chs=CHS, cho=CHO):
                for c in range(len(chs)):
                    if k < cho[c] + chs[c]:
                        return chunks[c][:, k - cho[c], cols]
                raise AssertionError

            with (
                tc.tile_pool(name="p1w", bufs=1) as p1w,
                tc.tile_pool(name="p1x", bufs=2) as p1x,
                tc.tile_pool(name="p1s", bufs=6) as p1s,
                tc.tile_pool(name="p1ps", bufs=4, space="PSUM") as p1ps,
            ):
                wchunks = {"q": [], "k": [], "v": []}

                def load_w(name, w_d, c, eng):
                    w = p1w.tile([128, CHS[c], JPC], BF16, tag=f"w{name}{c}")
                    eng.dma_start(
                        out=w[:], in_=w_d.ap()[:, CHO[c] : CHO[c] + CHS[c], :]
                    )
                    wchunks[name].append(w)

                def load_xs(s, engs=None):
                    xsc = []
                    for c in range(NCHX):
                        eng = nc.scalar if engs is None else engs[c]
                        xt = p1x.tile([128, CHSX[c], SB], BF16, tag=f"xs{c}")
                        eng.dma_start(
                            out=xt[:],
                            in_=x_d.ap()[s, :, CHOX[c] : CHOX[c] + CHSX[c], :],
                        )
                        xsc.append(xt)
                    return xsc

                # startup ordering: j0's operands (wq + x0) split across both
                # DMA rings so they arrive at aggregate HBM bandwidth; x0's
                # k8-11 chunk rides the sync ring behind wq to balance bytes.
                # (Further shuffles tested flat: the early phase is pinned by
                # HBM bandwidth + DGE outstanding-descriptor windows.)
                for c in range(NCH):
                    load_w("q", wq_d, c, nc.sync)
                xs_next = load_xs(
                    0,
                    (
                        nc.scalar,
                        nc.scalar,
                        nc.scalar,
                        nc.scalar,
                        nc.sync,
                        nc.scalar,
                    ),
                )
                # cos/sin for s-block 0 only (tiny), rest after wv
                sb0 = slice(0, SB)
                nc.scalar.dma_start(out=cos_sb[:, sb0], in_=cos_d.ap()[:, sb0])
                nc.scalar.dma_start(out=sin_sb[:, sb0], in_=sin_d.ap()[:, sb0])
                for c in range(NCH):
                    load_w("k", wk_d, c, nc.sync)
                for c in range(NCH):
                    load_w("v", wv_d, c, nc.scalar)
                rest = slice(SB, S)
                nc.scalar.dma_start(out=cos_sb[:, rest], in_=cos_d.ap()[:, rest])
                nc.scalar.dma_start(out=sin_sb[:, rest], in_=sin_d.ap()[:, rest])

                # s=3 interleaves q/k by head so qh[0]/kh[0] land early for
                # phase 2's first attention block
                order_qkv = [("q", j) for j in range(HPC)]
                order_qkv += [("k", j) for j in range(HPC)]
                order_qkv += [("v", j) for j in range(HPC)]
                order_int = []
                for j in range(HPC):
                    order_int += [("q", j), ("k", j)]
                order_int += [("v", j) for j in range(HPC)]

                for s in range(NSB):
                    sblk = slice(s * SB, (s + 1) * SB)
                    xsc = xs_next
                    if s + 1 < NSB:
                        # prefetch on the sync ring: FIFO order naturally
                        # deprioritizes it behind the critical weight loads
                        xs_next = load_xs(s + 1, (nc.sync,) * NCHX)
                    for name, j in order_int if s == NSB - 1 else order_qkv:
                        dst = {"q": qh, "k": kh, "v": None}[name]
                        jblk = slice(j * 128, (j + 1) * 128)
                        ps = p1ps.tile([128, SB], F32, tag="ps")
                        for k in range(KT):
                            if dst is not None:  # Q/K: [j, s] transposed
                                lhsT = wslice(wchunks[name], k, jblk)
                                rhs = wslice(xsc, k, slice(0, SB), CHSX, CHOX)
                            else:  # V: natural [s, j]
                                lhsT = wslice(xsc, k, jblk, CHSX, CHOX)
                                rhs = wslice(wchunks[name], k, slice(0, JPC))
                            nc.tensor.matmul(
                                ps[:],
                                lhsT,
                                rhs,
                                start=(k == 0),
                                stop=(k == KT - 1),
                            )
                        if dst is not None:
                            qt = p1s.tile([128, SB], BF16, tag="qt")
                            tmp = p1s.tile([128, SB], BF16, tag="tmp")
                            nc.scalar.copy(qt[:], ps[:])
                            # rotate-half swaps are SBUF->SBUF: issue them on
                            # the idle gpsimd DMA queue so x-prefetch issue
                            # slices on the sync ring cannot delay the RoPE
                            # chain (which backpressures PE via tile pools)
                            nc.gpsimd.dma_start(
                                out=tmp[0:64, :], in_=qt[64:128, :]
                            )
                            nc.gpsimd.dma_start(
                                out=tmp[64:128, :], in_=qt[0:64, :]
                            )
                            nc.vector.tensor_mul(qt[:], qt[:], cos_sb[:, sblk])
                            nc.vector.tensor_mul(tmp[:], tmp[:], sin_sb[:, sblk])
                            nc.vector.tensor_add(dst[j][:, sblk], qt[:], tmp[:])
                        else:
                            nc.scalar.copy(vnat[s * HPC + j][:], ps[:])

            # -------- Phase 2+3: attention with fused o_proj ---------------
            # n-outer block order: after column n's 4 heads, yt[:, :, nblk]
            # is complete, so column n's o_proj groups (4 matmuls each) are
            # interleaved into column n+1's attention blocks — they fill the
            # PE idle slots while ScalarE runs exp. PSUM: scores 4 + num 2 +
            # den 1 + o_proj 1 = 8 banks. The last column's o_proj runs after
            # phase 2 with all banks free.
            NOUT = HIDDEN // SB
            PIPE = 2
            NP = NKT // 2  # 8 score pairs

            def p3_group(m, nn, psp, ocp, ci):
                mblk = slice(m * 128, (m + 1) * 128)
                nblk = slice(nn * SB, (nn + 1) * SB)
                ps = psp.tile([128, SB], F32, tag="p3ps", name="p3ps")
                for kj in range(HPC):
                    nc.tensor.matmul(
                        ps[:],
                        yt[:, kj, mblk],
                        wo[:, kj, nblk],
                        start=(kj == 0),
                        stop=(kj == HPC - 1),
                    )
                oc = ocp.tile([128, SB], F32, tag="oc", name="oc")
                if ci % 2 == 0:
                    nc.vector.tensor_copy(oc[:], ps[:])
                else:
                    nc.scalar.copy(oc[:], ps[:])
                eng = nc.sync if ci % 2 == 0 else nc.scalar
                eng.dma_start(out=out[mblk, nblk], in_=oc[:])

            with (
                tc.tile_pool(name="p2e", bufs=6) as p2e,
                tc.tile_pool(name="p2ac", bufs=3) as p2ac,
                tc.tile_pool(name="p2t", bufs=3) as p2t,
                tc.tile_pool(name="p2r", bufs=3) as p2r,
                tc.tile_pool(name="p3s", bufs=6) as p3s,
                tc.tile_pool(name="p2sc", bufs=2, space="PSUM") as p2sc,
                tc.tile_pool(name="p2num", bufs=2, space="PSUM") as p2num,
                tc.tile_pool(name="p2den", bufs=1, space="PSUM") as p2den,
                tc.tile_pool(name="p3ps", bufs=1, space="PSUM") as p3ps,
            ):
                pending = None  # (acc, num, den, h, nblk) of previous block

                def flush_pending():
                    acc, num, den, ph, pnblk, w = pending
                    nc.tensor.matmul(
                        den[:, :w], ones_sb[:], acc[:, :w], start=True, stop=True
                    )
                    r = p2r.tile([128, SB], F32, tag="r")
                    scr = p2r.tile([128, SB], F32, tag="scr")
                    nc.vector.reciprocal_approx_accurate(
                        out=r[:, :w], in_=den[:, :w], scratch=scr[:, :w]
                    )
                    nc.vector.tensor_mul(yt[:, ph, pnblk], num[:, :w], r[:, :w])

                p3q = []  # (m, nn) o_proj groups ready to interleave
                p3n = [0]  # emitted-group counter (copy-engine round robin)

                # query-column segments: 3 full-width + 2 half-width. The
                # half-split of the last column lets its first half's o_proj
                # interleave into the second half's attention blocks, halving
                # the serial o_proj tail after phase 2.
                SEGS = (
                    (0, SB),
                    (SB, SB),
                    (2 * SB, SB),
                    (3 * SB, SB // 2),
                    (3 * SB + SB // 2, SB // 2),
                )
                for si, (no, w) in enumerate(SEGS):
                    nblk = slice(no, no + w)
                    if si >= 1:
                        po, pw = SEGS[si - 1]
                        p3q.extend(
                            (m, nn)
                            for m in range(po // 128, (po + pw) // 128)
                            for nn in range(NOUT)
                        )
                    for h in range(HPC):
                        hblk = slice(h * 128, (h + 1) * 128)
                        if si == 0 and h == 0:
                            for kj in range(HPC):
                                nc.sync.dma_start(
                                    out=wo[:, kj, :], in_=wo_d.ap()[:, kj, :]
                                )
                        num = p2num.tile([128, SB], F32, tag="num")
                        den = p2den.tile([128, SB], F32, tag="den")
                        acc = p2ac.tile([128, SB], BF16, tag="acc")
                        es = [None] * NP
                        for p in range(NP + PIPE):
                            if p < NP:
                                sc2 = p2sc.tile([128, 2, SB], F32, tag="sc")
                                e2 = p2e.tile([128, 2, SB], BF16, tag="e")
                                for half in range(2):
                                    i = 2 * p + half
                                    nc.tensor.matmul(
                                        sc2[:, half, :w],
                                        kh[h][:, i * 128 : (i + 1) * 128],
                                        qh[h][:, nblk],
                                        start=True,
                                        stop=True,
                                    )
                                nc.scalar.activation(
                                    e2[:, :, :w],
                                    sc2[:, :, :w],
                                    AF.Exp,
                                    scale=float(SCALE),
                                )
                                es[p] = e2
                                if p == 0:
                                    nc.vector.tensor_add(
                                        acc[:, :w], e2[:, 0, :w], e2[:, 1, :w]
                                    )
                                else:
                                    t2 = p2t.tile([128, SB], BF16, tag="t")
                                    nc.vector.tensor_add(
                                        t2[:, :w], e2[:, 0, :w], e2[:, 1, :w]
                                    )
                                    nc.vector.tensor_add(
                                        acc[:, :w], acc[:, :w], t2[:, :w]
                                    )
                            if p == PIPE + 1 and pending is not None:
                                # previous block's den matmul + normalize,
                                # emitted late so PE never waits on the DVE
                                # esum chain
                                flush_pending()
                            if p >= PIPE:
                                pp = p - PIPE
                                for half in range(2):
                                    i = 2 * pp + half
                                    nc.tensor.matmul(
                                        num[:, :w],
                                        vnat[i][:, hblk],
                                        es[pp][:, half, :w],
                                        start=(i == 0),
                                        stop=(i == NKT - 1),
                                    )
                            if p >= PIPE + 2 and p3q:
                                # one o_proj group of the previous column per
                                # slot; its matmuls fill PE idle under exp
                                p3_group(*p3q.pop(0), p3ps, p3s, p3n[0])
                                p3n[0] += 1
                        pending = (acc, num, den, h, nblk, w)
                flush_pending()

            # last half-segment's o_proj: phase-2 PSUM pools are closed, use
            # dense kj-outer groups across 4 banks with overlapped evacuation
            with (
                tc.tile_pool(name="p3sb", bufs=2) as p3sb,
                tc.tile_pool(name="p3psb", bufs=2, space="PSUM") as p3psb,
            ):
                for mi, m in enumerate(range(S // 128 - 2, S // 128)):
                    mblk = slice(m * 128, (m + 1) * 128)
                    ps4 = [
                        p3psb.tile([128, SB], F32, tag=f"ps{nn}", name=f"ps{nn}")
                        for nn in range(NOUT)
                    ]
                    if mi < 1:
                        for kj in range(HPC):
                            for nn in range(NOUT):
                                nblk = slice(nn * SB, (nn + 1) * SB)
                                nc.tensor.matmul(
                                    ps4[nn][:],
                                    yt[:, kj, mblk],
                                    wo[:, kj, nblk],
                                    start=(kj == 0),
                                    stop=(kj == HPC - 1),
                                )
                        for nn in range(NOUT):
                            nblk = slice(nn * SB, (nn + 1) * SB)
                            oc = p3sb.tile(
                                [128, SB], F32, tag=f"oc{nn}", name=f"oc{nn}"
                            )
                            nc.vector.tensor_copy(oc[:], ps4[nn][:])
                            eng = nc.sync if nn % 2 == 0 else nc.scalar
                            eng.dma_start(out=out[mblk, nblk], in_=oc[:])
                    else:
                        # last block: nn-outer so each PSUM tile finishes
                        # early and its copy/DMA overlaps remaining matmuls
                        for nn in range(NOUT):
                            nblk = slice(nn * SB, (nn + 1) * SB)
                            for kj in range(HPC):
                                nc.tensor.matmul(
                                    ps4[nn][:],
                                    yt[:, kj, mblk],
                                    wo[:, kj, nblk],
                                    start=(kj == 0),
                                    stop=(kj == HPC - 1),
                                )
                            oc = p3sb.tile(
                                [128, SB], F32, tag=f"oc{nn}", name=f"oc{nn}"
                            )
                            nc.vector.tensor_copy(oc[:], ps4[nn][:])
                            eng = nc.sync if nn % 2 == 0 else nc.scalar
                            eng.dma_start(out=out[mblk, nblk], in_=oc[:])

    nc.compile()
    return nc


_NC = None


def _get_nc():
    global _NC
    if _NC is None:
        _NC = build()
    return _NC


BF16_NP = np.dtype(mybir.dt.np(BF16))


def _rope_tables():
    inv_freq = 1.0 / (THETA ** (np.arange(0, DH, 2, dtype=np.float32) / DH))
    freqs = np.arange(S, dtype=np.float32)[:, None] * inv_freq[None, :]  # [S, 64]
    cos_h = np.cos(freqs).T.astype(np.float32)  # [64, S]
    sin_h = np.sin(freqs).T.astype(np.float32)
    cos = np.concatenate([cos_h, cos_h], axis=0)  # [128, S]
    sin_s = np.concatenate([-sin_h, sin_h], axis=0)  # [128, S]
    return (
        np.ascontiguousarray(cos).astype(BF16_NP),
        np.ascontiguousarray(sin_s).astype(BF16_NP),
    )


def _pm_weight(wT):  # [2048, 512] (k, j) -> [128, 16, 512] partition-major
    return np.ascontiguousarray(
        wT.reshape(KT, 128, JPC).transpose(1, 0, 2)
    ).astype(BF16_NP)


def _make_in_maps(inputs):
    x = np.asarray(inputs["x"], dtype=np.float32)
    Wq = np.asarray(inputs["Wq"], dtype=np.float32)
    Wk = np.asarray(inputs["Wk"], dtype=np.float32)
    Wv = np.asarray(inputs["Wv"], dtype=np.float32)
    Wo = np.asarray(inputs["Wo"], dtype=np.float32)

    cos, sin_s = _rope_tables()

    in_maps = []
    for c in range(N_CORES):
        b = c // GROUPS
        g = c % GROUPS
        rows = slice(g * JPC, (g + 1) * JPC)
        xT = x[b].T  # [hidden(k), s]
        # [k, s] -> [s_blk, p, kt, s_in_blk]
        xpm = np.ascontiguousarray(
            xT.reshape(KT, 128, NSB, SB).transpose(2, 1, 0, 3)
        ).astype(BF16_NP)
        # Wo[:, rows].T -> [512(j), 2048] -> [p, kj, 2048]
        woT = Wo[:, rows].T
        wopm = np.ascontiguousarray(
            woT.reshape(HPC, 128, HIDDEN).transpose(1, 0, 2)
        ).astype(BF16_NP)
        in_maps.append(
            {
                "xPM": xpm,
                "wqPM": _pm_weight(Wq[rows].T),
                "wkPM": _pm_weight(Wk[rows].T),
                "wvPM": _pm_weight(Wv[rows].T),
                "woPM": wopm,
                "cos": cos,
                "sinS": sin_s,
            }
        )
    return in_maps


def kernel(x, Wq, Wk, Wv, Wo):
    nc = _get_nc()
    in_maps = _make_in_maps({"x": x, "Wq": Wq, "Wk": Wk, "Wv": Wv, "Wo": Wo})
    res = run_bass_kernel_spmd(nc, in_maps, list(range(N_CORES)))

    out = np.zeros((B, S, HIDDEN), dtype=np.float32)
    for c in range(N_CORES):
        out[c // GROUPS] += res.results[c]["out"]
    return out


# revision 51
# speedup vs baseline: 1.0242x; 1.0242x over previous
"""Trainium2 Bass kernel for nn_BaseTimeAttention (dense transformer block:
QKV projection + RoPE + softmax attention + output projection).

Problem (hardcoded):
  x:  [B=2, S=2048, H=2048] fp32,  Wq/Wk/Wv/Wo: [2048, 2048] fp32
  out = softmax((rope(xWq^T) rope(xWk^T)^T)/sqrt(128)) (xWv^T) Wo^T

Sharding (8 cores): tensor-parallel over heads x data-parallel over batch.
Core c handles batch b=c//4 and head group g=c%4 (4 of 16 heads = 512 of 2048
channels). Each core produces a full [2048, 2048] partial of the output
projection restricted to its 512 input channels; the host sums 4 partials per
batch (o_proj row-parallel reduction on host).

v2 design (vs the fp32r baseline at 532us):
  * Everything flows in bf16 except PSUM accumulation and the final output.
    Halves all DMA traffic and lets q/k/v stay SBUF-resident for the whole
    kernel -- no DRAM round trip between projection and attention, so the
    phase transitions have no DMA stall.
  * Softmax denominator: instead of 16 ones-matmuls per (head, q-block)
    (256 matmuls, ~60us of PE), the exp tiles are accumulated on the Vector
    engine (bf16 adds) and a single ones-matmul per block contracts the
    128 partitions. The den matmul + normalize for block B is emitted at the
    start of block B+1 so the PE never waits on the DVE accumulation chain.
  * Startup DMAs are ordered so the first matmul can start ~2us in: first
    weight chunk on the sync ring || first x chunk then per-block cos/sin
    slices on the scalar ring.

Per-core PE work: 768 (proj) + 256 (scores) + 256 (attn@V) + 16 (den) +
256 (o_proj) = 1552 matmuls of 512 moving rows = 795k cycles ~= 331us at
2.4GHz; phase 2 is co-limited by ScalarE exp (~140us for the 16.8M scores).
"""

import numpy as np

import concourse.mybir as mybir
import concourse.tile as tile
from concourse import bacc
from concourse.bass_utils import run_bass_kernel_spmd

F32 = mybir.dt.float32
BF16 = mybir.dt.bfloat16
F8 = mybir.dt.float8e4
AF = mybir.ActivationFunctionType

B = 2
S = 2048
HIDDEN = 2048
HEADS = 16
DH = 128
THETA = 10000.0
N_CORES = 8
GROUPS = 4
HPC = HEADS // GROUPS  # heads per core
JPC = HPC * DH  # projection cols per core
SCALE = 1.0 / np.sqrt(DH)

SB = 512
NSB = S // SB
KT = HIDDEN // 128  # 16 contraction tiles
NKT = S // 128  # 16 s_k tiles


def build():
    nc = bacc.Bacc("TRN2", target_bir_lowering=False, debug=False)

    # partition-major inputs (see _make_in_maps)
    x_d = nc.dram_tensor("xPM", [NSB, 128, KT, SB], BF16, kind="ExternalInput")
    wq_d = nc.dram_tensor("wqPM", [128, KT, JPC], BF16, kind="ExternalInput")
    wk_d = nc.dram_tensor("wkPM", [128, KT, JPC], BF16, kind="ExternalInput")
    wv_d = nc.dram_tensor("wvPM", [128, KT, JPC], BF16, kind="ExternalInput")
    wo_d = nc.dram_tensor("woPM", [128, HPC, HIDDEN], BF16, kind="ExternalInput")
    cos_d = nc.dram_tensor("cos", [DH, S], BF16, kind="ExternalInput")
    sin_d = nc.dram_tensor("sinS", [DH, S], BF16, kind="ExternalInput")
    out_d = nc.dram_tensor("out", [S, HIDDEN], F32, kind="ExternalOutput")

    out = out_d.ap()

    with tile.TileContext(nc) as tc:
        with tc.tile_pool(name="persist", bufs=1) as persist:
            ones_sb = persist.tile([128, 128], BF16, tag="ones")
            nc.gpsimd.memset(ones_sb[:], 1.0)

            # SBUF-resident per-head q/k (transposed [dh, s]) and natural v
            qh = [
                persist.tile([128, S], BF16, tag=f"qh{h}", name=f"qh{h}")
                for h in range(HPC)
            ]
            kh = [
                persist.tile([128, S], BF16, tag=f"kh{h}", name=f"kh{h}")
                for h in range(HPC)
            ]
            vnat = [
                persist.tile([128, JPC], BF16, tag=f"v{t}", name=f"v{t}")
                for t in range(NKT)
            ]
            cos_sb = persist.tile([128, S], BF16, tag="cos")
            sin_sb = persist.tile([128, S], BF16, tag="sin")
            wo = persist.tile([128, HPC, HIDDEN], BF16, tag="wo")
            yt = persist.tile([128, HPC, S], BF16, tag="yt")

            # ---------------- Phase 1: projections + RoPE ------------------
            # s-block outer, projection inner: x is streamed from HBM once.
            # Chunked loads (finest first) so the first matmuls start as soon
            # as the DMA rings deliver data (~10us fixed startup latency).
            NCH = 5
            CHS = (1, 1, 2, 4, 8)  # weight chunk sizes
            CHO = (0, 1, 2, 4, 8)  # weight chunk offsets
            NCHX = 6
            CHSX = (1, 1, 2, 4, 4, 4)  # x chunk sizes (k8-11 ride sync ring)
            CHOX = (0, 1, 2, 4, 8, 12)

            def wslice(chunks, k, cols, chs=CHS, cho=CHO):
                for c in range(len(chs)):
                    if k < cho[c] + chs[c]:
                        return chunks[c][:, k - cho[c], cols]
                raise AssertionError

            with (
                tc.tile_pool(name="p1w", bufs=1) as p1w,
                tc.tile_pool(name="p1x", bufs=2) as p1x,
                tc.tile_pool(name="p1s", bufs=6) as p1s,
                tc.tile_pool(name="p1ps", bufs=4, space="PSUM") as p1ps,
            ):
                wchunks = {"q": [], "k": [], "v": []}

                def load_w(name, w_d, c, eng):
                    w = p1w.tile([128, CHS[c], JPC], BF16, tag=f"w{name}{c}")
                    eng.dma_start(
                        out=w[:], in_=w_d.ap()[:, CHO[c] : CHO[c] + CHS[c], :]
                    )
                    wchunks[name].append(w)

                def load_xs(s, engs=None):
                    xsc = []
                    for c in range(NCHX):
                        eng = nc.scalar if engs is None else engs[c]
                        xt = p1x.tile([128, CHSX[c], SB], BF16, tag=f"xs{c}")
                        eng.dma_start(
                            out=xt[:],
                            in_=x_d.ap()[s, :, CHOX[c] : CHOX[c] + CHSX[c], :],
                        )
                        xsc.append(xt)
                    return xsc

                # startup ordering: j0's operands (wq + x0) split across both
                # DMA rings so they arrive at aggregate HBM bandwidth; x0's
                # k8-11 chunk rides the sync ring behind wq to balance bytes.
                # (Further shuffles tested flat: the early phase is pinned by
                # HBM bandwidth + DGE outstanding-descriptor windows.)
                for c in range(NCH):
                    load_w("q", wq_d, c, nc.sync)
                xs_next = load_xs(
                    0,
                    (
                        nc.scalar,
                        nc.scalar,
                        nc.scalar,
                        nc.scalar,
                        nc.sync,
                        nc.scalar,
                    ),
                )
                # cos/sin for s-block 0 only (tiny), rest after wv
                sb0 = slice(0, SB)
                nc.scalar.dma_start(out=cos_sb[:, sb0], in_=cos_d.ap()[:, sb0])
                nc.scalar.dma_start(out=sin_sb[:, sb0], in_=sin_d.ap()[:, sb0])
                for c in range(NCH):
                    load_w("k", wk_d, c, nc.sync)
                for c in range(NCH):
                    load_w("v", wv_d, c, nc.scalar)
                rest = slice(SB, S)
                nc.scalar.dma_start(out=cos_sb[:, rest], in_=cos_d.ap()[:, rest])
                nc.scalar.dma_start(out=sin_sb[:, rest], in_=sin_d.ap()[:, rest])

                # s=3 interleaves q/k by head so qh[0]/kh[0] land early for
                # phase 2's first attention block
                order_qkv = [("q", j) for j in range(HPC)]
                order_qkv += [("k", j) for j in range(HPC)]
                order_qkv += [("v", j) for j in range(HPC)]
                order_int = []
                for j in range(HPC):
                    order_int += [("q", j), ("k", j)]
                order_int += [("v", j) for j in range(HPC)]

                for s in range(NSB):
                    sblk = slice(s * SB, (s + 1) * SB)
                    xsc = xs_next
                    if s + 1 < NSB:
                        # prefetch on the sync ring: FIFO order naturally
                        # deprioritizes it behind the critical weight loads
                        xs_next = load_xs(s + 1, (nc.sync,) * NCHX)
                    for name, j in order_int if s == NSB - 1 else order_qkv:
                        dst = {"q": qh, "k": kh, "v": None}[name]
                        jblk = slice(j * 128, (j + 1) * 128)
                        ps = p1ps.tile([128, SB], F32, tag="ps")
                        for k in range(KT):
                            if dst is not None:  # Q/K: [j, s] transposed
                                lhsT = wslice(wchunks[name], k, jblk)
                                rhs = wslice(xsc, k, slice(0, SB), CHSX, CHOX)
                            else:  # V: natural [s, j]
                                lhsT = wslice(xsc, k, jblk, CHSX, CHOX)
                                rhs = wslice(wchunks[name], k, slice(0, JPC))
                            nc.tensor.matmul(
                                ps[:],
                                lhsT,
                                rhs,
                                start=(k == 0),
                                stop=(k == KT - 1),
                            )
                        if dst is not None:
                            qt = p1s.tile([128, SB], BF16, tag="qt")
                            tmp = p1s.tile([128, SB], BF16, tag="tmp")
                            nc.scalar.copy(qt[:], ps[:])
                            # rotate-half swaps are SBUF->SBUF: issue them on
                            # the idle gpsimd DMA queue so x-prefetch issue
                            # slices on the sync ring cannot delay the RoPE
                            # chain (which backpressures PE via tile pools)
                            nc.gpsimd.dma_start(
                                out=tmp[0:64, :], in_=qt[64:128, :]
                            )
                            nc.gpsimd.dma_start(
                                out=tmp[64:128, :], in_=qt[0:64, :]
                            )
                            nc.vector.tensor_mul(qt[:], qt[:], cos_sb[:, sblk])
                            nc.vector.tensor_mul(tmp[:], tmp[:], sin_sb[:, sblk])
                            nc.vector.tensor_add(dst[j][:, sblk], qt[:], tmp[:])
                        else:
                            nc.scalar.copy(vnat[s * HPC + j][:], ps[:])

            # -------- Phase 2+3: attention with fused o_proj ---------------
            # n-outer block order: after column n's 4 heads, yt[:, :, nblk]
            # is complete, so column n's o_proj groups (4 matmuls each) are
            # interleaved into column n+1's attention blocks — they fill the
            # PE idle slots while ScalarE runs exp. PSUM: scores 4 + num 2 +
            # den 1 + o_proj 1 = 8 banks. The last column's o_proj runs after
            # phase 2 with all banks free.
            NOUT = HIDDEN // SB
            PIPE = 2
            NP = NKT // 2  # 8 score pairs

            def p3_group(m, nn, psp, ocp, ci):
                mblk = slice(m * 128, (m + 1) * 128)
                nblk = slice(nn * SB, (nn + 1) * SB)
                ps = psp.tile([128, SB], F32, tag="p3ps", name="p3ps")
                for kj in range(HPC):
                    nc.tensor.matmul(
                        ps[:],
                        yt[:, kj, mblk],
                        wo[:, kj, nblk],
                        start=(kj == 0),
                        stop=(kj == HPC - 1),
                    )
                oc = ocp.tile([128, SB], F32, tag="oc", name="oc")
                if ci % 2 == 0:
                    nc.vector.tensor_copy(oc[:], ps[:])
                else:
                    nc.scalar.copy(oc[:], ps[:])
                eng = nc.sync if ci % 2 == 0 else nc.scalar
                eng.dma_start(out=out[mblk, nblk], in_=oc[:])

            with (
                tc.tile_pool(name="p2e", bufs=6) as p2e,
                tc.tile_pool(name="p2ac", bufs=3) as p2ac,
                tc.tile_pool(name="p2t", bufs=3) as p2t,
                tc.tile_pool(name="p2r", bufs=3) as p2r,
                tc.tile_pool(name="p3s", bufs=6) as p3s,
                tc.tile_pool(name="p2sc", bufs=2, space="PSUM") as p2sc,
                tc.tile_pool(name="p2num", bufs=2, space="PSUM") as p2num,
                tc.tile_pool(name="p2den", bufs=1, space="PSUM") as p2den,
                tc.tile_pool(name="p3ps", bufs=1, space="PSUM") as p3ps,
            ):
                pending = None  # (acc, num, den, h, nblk) of previous block

                def flush_pending():
                    acc, num, den, ph, pnblk = pending
                    nc.tensor.matmul(
                        den[:], ones_sb[:], acc[:], start=True, stop=True
                    )
                    r = p2r.tile([128, SB], F32, tag="r")
                    scr = p2r.tile([128, SB], F32, tag="scr")
                    nc.vector.reciprocal_approx_accurate(
                        out=r[:], in_=den[:], scratch=scr[:]
                    )
                    nc.vector.tensor_mul(yt[:, ph, pnblk], num[:], r[:])

                p3q = []  # (m, nn) o_proj groups ready to interleave
                p3n = [0]  # emitted-group counter (copy-engine round robin)

                for n in range(NSB):
                    nblk = slice(n * SB, (n + 1) * SB)
                    if n >= 1:
                        p3q.extend(
                            (m, nn)
                            for m in range(HPC * (n - 1), HPC * n)
                            for nn in range(NOUT)
                        )
                    for h in range(HPC):
                        hblk = slice(h * 128, (h + 1) * 128)
                        if n == 0 and h == 0:
                            for kj in range(HPC):
                                nc.sync.dma_start(
                                    out=wo[:, kj, :], in_=wo_d.ap()[:, kj, :]
                                )
                        num = p2num.tile([128, SB], F32, tag="num")
                        den = p2den.tile([128, SB], F32, tag="den")
                        acc = p2ac.tile([128, SB], BF16, tag="acc")
                        es = [None] * NP
                        for p in range(NP + PIPE):
                            if p < NP:
                                sc2 = p2sc.tile([128, 2, SB], F32, tag="sc")
                                e2 = p2e.tile([128, 2, SB], BF16, tag="e")
                                for half in range(2):
                                    i = 2 * p + half
                                    nc.tensor.matmul(
                                        sc2[:, half, :],
                                        kh[h][:, i * 128 : (i + 1) * 128],
                                        qh[h][:, nblk],
                                        start=True,
                                        stop=True,
                                    )
                                nc.scalar.activation(
                                    e2[:], sc2[:], AF.Exp, scale=float(SCALE)
                                )
                                es[p] = e2
                                if p == 0:
                                    nc.vector.tensor_add(
                                        acc[:], e2[:, 0, :], e2[:, 1, :]
                                    )
                                else:
                                    t2 = p2t.tile([128, SB], BF16, tag="t")
                                    nc.vector.tensor_add(
                                        t2[:], e2[:, 0, :], e2[:, 1, :]
                                    )
                                    nc.vector.tensor_add(
                                        acc[:], acc[:], t2[:]
                                    )
                            if p == PIPE + 1 and pending is not None:
                                # previous block's den matmul + normalize,
                                # emitted late so PE never waits on the DVE
                                # esum chain
                                flush_pending()
                            if p >= PIPE:
                                pp = p - PIPE
                                for half in range(2):
                                    i = 2 * pp + half
                                    nc.tensor.matmul(
                                        num[:],
                                        vnat[i][:, hblk],
                                        es[pp][:, half, :],
                                        start=(i == 0),
                                        stop=(i == NKT - 1),
                                    )
                            if p >= PIPE + 2 and p3q:
                                # one o_proj group of the previous column per
                                # slot; its matmuls fill PE idle under exp
                                p3_group(*p3q.pop(0), p3ps, p3s, p3n[0])
                                p3n[0] += 1
                        pending = (acc, num, den, h, nblk)
                flush_pending()

            # last column's o_proj: phase-2 PSUM pools are closed, use dense
            # kj-outer groups across 4 banks with overlapped evacuation
            with (
                tc.tile_pool(name="p3sb", bufs=2) as p3sb,
                tc.tile_pool(name="p3psb", bufs=2, space="PSUM") as p3psb,
            ):
                for mi, m in enumerate(range(S // 128 - HPC, S // 128)):
                    mblk = slice(m * 128, (m + 1) * 128)
                    ps4 = [
                        p3psb.tile([128, SB], F32, tag=f"ps{nn}", name=f"ps{nn}")
                        for nn in range(NOUT)
                    ]
                    if mi < HPC - 1:
                        for kj in range(HPC):
                            for nn in range(NOUT):
                                nblk = slice(nn * SB, (nn + 1) * SB)
                                nc.tensor.matmul(
                                    ps4[nn][:],
                                    yt[:, kj, mblk],
                                    wo[:, kj, nblk],
                                    start=(kj == 0),
                                    stop=(kj == HPC - 1),
                                )
                        for nn in range(NOUT):
                            nblk = slice(nn * SB, (nn + 1) * SB)
                            oc = p3sb.tile(
                                [128, SB], F32, tag=f"oc{nn}", name=f"oc{nn}"
                            )
                            nc.vector.tensor_copy(oc[:], ps4[nn][:])
                            eng = nc.sync if nn % 2 == 0 else nc.scalar
                            eng.dma_start(out=out[mblk, nblk], in_=oc[:])
                    else:
                        # last block: nn-outer so each PSUM tile finishes
                        # early and its copy/DMA overlaps remaining matmuls
                        for nn in range(NOUT):
                            nblk = slice(nn * SB, (nn + 1) * SB)
                            for kj in range(HPC):
                                nc.tensor.matmul(
                                    ps4[nn][:],
                                    yt[:, kj, mblk],
                                    wo[:, kj, nblk],
                                    start=(kj == 0),
                                    stop=(kj == HPC - 1),
                                )
                            oc = p3sb.tile(
                                [128, SB], F32, tag=f"oc{nn}", name=f"oc{nn}"
                            )
                            nc.vector.tensor_copy(oc[:], ps4[nn][:])
                            eng = nc.sync if nn % 2 == 0 else nc.scalar
                            eng.dma_start(out=out[mblk, nblk], in_=oc[:])

    nc.compile()
    return nc


_NC = None


def _get_nc():
    global _NC
    if _NC is None:
        _NC = build()
    return _NC


BF16_NP = np.dtype(mybir.dt.np(BF16))


def _rope_tables():
    inv_freq = 1.0 / (THETA ** (np.arange(0, DH, 2, dtype=np.float32) / DH))
    freqs = np.arange(S, dtype=np.float32)[:, None] * inv_freq[None, :]  # [S, 64]
    cos_h = np.cos(freqs).T.astype(np.float32)  # [64, S]
    sin_h = np.sin(freqs).T.astype(np.float32)
    cos = np.concatenate([cos_h, cos_h], axis=0)  # [128, S]
    sin_s = np.concatenate([-sin_h, sin_h], axis=0)  # [128, S]
    return (
        np.ascontiguousarray(cos).astype(BF16_NP),
        np.ascontiguousarray(sin_s).astype(BF16_NP),
    )


def _pm_weight(wT):  # [2048, 512] (k, j) -> [128, 16, 512] partition-major
    return np.ascontiguousarray(
        wT.reshape(KT, 128, JPC).transpose(1, 0, 2)
    ).astype(BF16_NP)


def _make_in_maps(inputs):
    x = np.asarray(inputs["x"], dtype=np.float32)
    Wq = np.asarray(inputs["Wq"], dtype=np.float32)
    Wk = np.asarray(inputs["Wk"], dtype=np.float32)
    Wv = np.asarray(inputs["Wv"], dtype=np.float32)
    Wo = np.asarray(inputs["Wo"], dtype=np.float32)

    cos, sin_s = _rope_tables()

    in_maps = []
    for c in range(N_CORES):
        b = c // GROUPS
        g = c % GROUPS
        rows = slice(g * JPC, (g + 1) * JPC)
        xT = x[b].T  # [hidden(k), s]
        # [k, s] -> [s_blk, p, kt, s_in_blk]
        xpm = np.ascontiguousarray(
            xT.reshape(KT, 128, NSB, SB).transpose(2, 1, 0, 3)
        ).astype(BF16_NP)
        # Wo[:, rows].T -> [512(j), 2048] -> [p, kj, 2048]
        woT = Wo[:, rows].T
        wopm = np.ascontiguousarray(
            woT.reshape(HPC, 128, HIDDEN).transpose(1, 0, 2)
        ).astype(BF16_NP)
        in_maps.append(
            {
                "xPM": xpm,
                "wqPM": _pm_weight(Wq[rows].T),
                "wkPM": _pm_weight(Wk[rows].T),
                "wvPM": _pm_weight(Wv[rows].T),
                "woPM": wopm,
                "cos": cos,
                "sinS": sin_s,
            }
        )
    return in_maps


def kernel(x, Wq, Wk, Wv, Wo):
    nc = _get_nc()
    in_maps = _make_in_maps({"x": x, "Wq": Wq, "Wk": Wk, "Wv": Wv, "Wo": Wo})
    res = run_bass_kernel_spmd(nc, in_maps, list(range(N_CORES)))

    out = np.zeros((B, S, HIDDEN), dtype=np.float32)
    for c in range(N_CORES):
        out[c // GROUPS] += res.results[c]["out"]
    return out
